# revision 3
# baseline (speedup 1.0000x reference)
"""2-layer GCN (GCNConv -> relu -> GCNConv -> log_softmax) on 8 NeuronCores.

Strategy (standard distributed GNN data parallel):
  - nodes partitioned into 8 contiguous shards; core c owns dst shard c
  - edges partitioned by dst owner; within a core, bucketed by
    (src-octant "group", dst-chunk) and sorted by dst
  - per-layer aggregation on device:
      table   = dis * (features)          [16 feature-partitions x 8 src-octant groups]
      gather  = gpsimd ap_gather (per-group index streams)
      segsum  = DVE segmented scan (mask resets at dst-run starts)
      extract = gpsimd ap_gather of run-end prefix values
      combine = PE matmul with a group-summing 0/1 matrix
  - self-loop term handled analytically (dis_d^2 * h), bias/relu/W2/log_softmax
    fused in the epilogue
  - halo exchange of the (tiny) transformed features between launches is done
    on the host (concat of per-core shard outputs)

All floating point math runs on device in fp32 (masks are exact 0/1 bf16).
Host only does integer graph preprocessing and data movement.
"""
import os
import sys

sys.path.insert(0, '/opt/trn_rl_repo')

import numpy as np
import ml_dtypes

from concourse import bass, bacc, mybir
import concourse.tile as tile
from concourse.masks import make_identity
from concourse.bass_utils import run_bass_kernel_spmd

F32 = mybir.dt.float32
BF16 = mybir.dt.bfloat16
I16 = mybir.dt.int16
I32 = mybir.dt.int32

NCORES = 8
NG = 8  # src-octant groups (16 partitions each)

# accumulated HW time of the launches of the most recent kernel() call
LAST_EXEC_NS = []

_cache = {}


def _cdiv(a, b):
    return (a + b - 1) // b


# ----------------------------------------------------------------- launch A
def _build_launch_a(SH_PAD, IN, HID):
    NTILE = SH_PAD // 128
    nc = bacc.Bacc("TRN2", target_bir_lowering=False, debug=False, num_devices=NCORES)
    xs_d = nc.dram_tensor("xs", [SH_PAD, IN], F32, kind="ExternalInput")
    deg_d = nc.dram_tensor("deg2d", [128, NTILE], I32, kind="ExternalInput")
    w1_d = nc.dram_tensor("w1", [IN, HID], F32, kind="ExternalInput")
    h1sT_d = nc.dram_tensor("h1sT", [HID, SH_PAD], F32, kind="ExternalOutput")
    dis_d = nc.dram_tensor("dis2d", [128, NTILE], F32, kind="ExternalOutput")

    with tile.TileContext(nc) as tc:
        with (
            tc.tile_pool(name="persist", bufs=1) as pp,
            tc.tile_pool(name="loop", bufs=3) as lp,
            tc.tile_pool(name="psum", bufs=3, space="PSUM") as psp,
            tc.tile_pool(name="psum2", bufs=2, space="PSUM") as psp2,
        ):
            ident = pp.tile([128, 128], F32)
            make_identity(nc, ident[:])
            w1 = pp.tile([IN, HID], F32)
            nc.sync.dma_start(out=w1[:], in_=w1_d[:])
            degt = pp.tile([128, NTILE], I32)
            nc.sync.dma_start(out=degt[:], in_=deg_d[:])
            degf = pp.tile([128, NTILE], F32)
            nc.vector.tensor_copy(out=degf[:], in_=degt[:])
            dis = pp.tile([128, NTILE], F32)
            nc.vector.reciprocal(out=dis[:], in_=degf[:])
            nc.scalar.activation(dis[:], dis[:], mybir.ActivationFunctionType.Sqrt)
            nc.sync.dma_start(out=dis_d[:], in_=dis[:])

            h1sT = pp.tile([HID, SH_PAD], F32)
            for t in range(NTILE):
                xt = lp.tile([128, IN], F32, tag="xt")
                nc.sync.dma_start(out=xt[:], in_=xs_d[t * 128:(t + 1) * 128, :])
                nc.vector.tensor_tensor(
                    out=xt[:], in0=xt[:],
                    in1=dis[:, t:t + 1].to_broadcast([128, IN]),
                    op=mybir.AluOpType.mult)
                pT = psp.tile([128, 128], F32, tag="pT")
                nc.tensor.transpose(out=pT[:, :IN], in_=xt[:], identity=ident[:])
                xT = lp.tile([IN, 128], F32, tag="xT")
                nc.scalar.copy(out=xT[:], in_=pT[:IN, :])
                ph = psp2.tile([HID, 128], F32, tag="ph")
                nc.tensor.matmul(out=ph[:], lhsT=w1[:], rhs=xT[:],
                                 start=True, stop=True)
                nc.vector.tensor_copy(out=h1sT[:, t * 128:(t + 1) * 128], in_=ph[:])
            nc.sync.dma_start(out=h1sT_d[:], in_=h1sT[:])
    nc.compile()
    return nc


# --------------------------------------------------------------- launch B/C
def _build_launch_agg(W, C, NCHUNK, DST_CH, DST_PAD, HID, OUT, layer):
    """layer=1: combine->+self->*dis->+b1->relu->W2->*dis -> z [2, DST_PAD]
    layer=2: combine->+self->*dis->+b2 -> log_softmax -> out [2, 128, SMR]"""
    F = HID if layer == 1 else OUT
    SMR = DST_PAD // 128
    nc = bacc.Bacc("TRN2", target_bir_lowering=False, debug=False, num_devices=NCORES)
    table_d = nc.dram_tensor("table", [128, W], F32, kind="ExternalInput")
    idx_d = nc.dram_tensor("idx", [128, NCHUNK * (C // 16)], I16, kind="ExternalInput")
    mask_d = nc.dram_tensor("mask", [128, NCHUNK * C], BF16, kind="ExternalInput")
    ext_d = nc.dram_tensor("ext", [128, NCHUNK * (DST_CH // 16)], I16, kind="ExternalInput")
    disr_d = nc.dram_tensor("disrep", [F, DST_PAD], F32, kind="ExternalInput")
    self_d = nc.dram_tensor("selfv", [F, DST_PAD], F32, kind="ExternalInput")
    bias_d = nc.dram_tensor("bias", [F, 1], F32, kind="ExternalInput")
    g_d = nc.dram_tensor("gmat", [128, F], F32, kind="ExternalInput")
    if layer == 1:
        w2_d = nc.dram_tensor("w2", [HID, OUT], F32, kind="ExternalInput")
        out_d = nc.dram_tensor("z", [OUT, DST_PAD], F32, kind="ExternalOutput")
    else:
        out_d = nc.dram_tensor("o", [OUT, 128, SMR], F32, kind="ExternalOutput")

    NS = DST_CH // 512  # 512-col epilogue slices per chunk

    with tile.TileContext(nc) as tc:
        with (
            tc.tile_pool(name="persist", bufs=1) as pp,
            tc.tile_pool(name="loop", bufs=2) as lp,
            tc.tile_pool(name="big", bufs=1) as bigp,
            tc.tile_pool(name="ep", bufs=3) as ep,
            tc.tile_pool(name="psA", bufs=2, space="PSUM") as psA,
            tc.tile_pool(name="psB", bufs=2, space="PSUM") as psB,
            tc.tile_pool(name="dram", bufs=1, space="DRAM") as dp,
        ):
            table = pp.tile([128, W], F32)
            nc.sync.dma_start(out=table[:], in_=table_d[:])
            gmat = pp.tile([128, F], F32)
            nc.sync.dma_start(out=gmat[:], in_=g_d[:])
            bias = pp.tile([F, 1], F32)
            nc.sync.dma_start(out=bias[:], in_=bias_d[:])
            if layer == 1:
                w2 = pp.tile([HID, OUT], F32)
                nc.sync.dma_start(out=w2[:], in_=w2_d[:])
            if layer == 2:
                z2 = dp.tile([OUT, DST_PAD], F32)

            for k in range(NCHUNK):
                idx_t = lp.tile([128, C // 16], I16, tag="idx")
                nc.sync.dma_start(out=idx_t[:], in_=idx_d[:, k * (C // 16):(k + 1) * (C // 16)])
                mask_t = lp.tile([128, C], BF16, tag="mask")
                nc.sync.dma_start(out=mask_t[:], in_=mask_d[:, k * C:(k + 1) * C])
                ext_t = lp.tile([128, DST_CH // 16], I16, tag="ext")
                nc.sync.dma_start(out=ext_t[:], in_=ext_d[:, k * (DST_CH // 16):(k + 1) * (DST_CH // 16)])
                self_t = lp.tile([F, DST_CH], F32, tag="selfv")
                nc.sync.dma_start(out=self_t[:], in_=self_d[:, k * DST_CH:(k + 1) * DST_CH])
                disr_t = lp.tile([F, DST_CH], F32, tag="disr")
                nc.sync.dma_start(out=disr_t[:], in_=disr_d[:, k * DST_CH:(k + 1) * DST_CH])

                msg = bigp.tile([128, C], F32, tag="msg")
                nc.gpsimd.ap_gather(
                    out_ap=msg[:], in_ap=table[:], idxs_ap=idx_t[:],
                    channels=128, num_elems=W, d=1, num_idxs=C)
                csum = bigp.tile([128, C + 16], F32, tag="csum")
                nc.vector.memset(csum[:, 0:1], 0.0)
                nc.vector.tensor_tensor_scan(
                    out=csum[:, 1:C + 1], data0=mask_t[:], data1=msg[:],
                    initial=0.0, op0=mybir.AluOpType.mult, op1=mybir.AluOpType.add)
                extv = lp.tile([128, DST_CH], F32, tag="extv")
                nc.gpsimd.ap_gather(
                    out_ap=extv[:], in_ap=csum[:, 0:C + 1], idxs_ap=ext_t[:],
                    channels=128, num_elems=C + 1, d=1, num_idxs=DST_CH)

                for s in range(NS):
                    sl = slice(s * 512, (s + 1) * 512)
                    ps = psA.tile([F, 512], F32, tag="ps")
                    nc.tensor.matmul(out=ps[:], lhsT=gmat[:], rhs=extv[:, sl],
                                     start=True, stop=True)
                    a1 = ep.tile([F, 512], F32, tag="a1")
                    nc.vector.tensor_tensor(out=a1[:], in0=ps[:], in1=self_t[:, sl],
                                            op=mybir.AluOpType.add)
                    nc.vector.tensor_tensor(out=a1[:], in0=a1[:], in1=disr_t[:, sl],
                                            op=mybir.AluOpType.mult)
                    nc.vector.tensor_tensor(out=a1[:], in0=a1[:],
                                            in1=bias[:].to_broadcast([F, 512]),
                                            op=mybir.AluOpType.add)
                    if layer == 1:
                        nc.vector.tensor_scalar_max(a1[:], a1[:], 0.0)
                        ps2 = psB.tile([OUT, 512], F32, tag="ps2")
                        nc.tensor.matmul(out=ps2[:], lhsT=w2[:], rhs=a1[:],
                                         start=True, stop=True)
                        zt = ep.tile([OUT, 512], F32, tag="zt")
                        nc.vector.tensor_tensor(out=zt[:], in0=ps2[:],
                                                in1=disr_t[:OUT, sl],
                                                op=mybir.AluOpType.mult)
                        nc.sync.dma_start(
                            out=out_d[:, k * DST_CH + s * 512:k * DST_CH + (s + 1) * 512],
                            in_=zt[:])
                    else:
                        nc.sync.dma_start(
                            out=z2[:, k * DST_CH + s * 512:k * DST_CH + (s + 1) * 512],
                            in_=a1[:])

            if layer == 2:
                # log_softmax over the 2 classes, done in [128, SMR] layout
                z0 = pp.tile([128, SMR], F32)
                z1 = pp.tile([128, SMR], F32)
                nc.sync.dma_start(out=z0[:], in_=z2[0:1, :].rearrange('o (p f) -> (o p) f', p=128))
                nc.sync.dma_start(out=z1[:], in_=z2[1:2, :].rearrange('o (p f) -> (o p) f', p=128))
                m = pp.tile([128, SMR], F32)
                nc.vector.tensor_tensor(out=m[:], in0=z0[:], in1=z1[:], op=mybir.AluOpType.max)
                d0 = pp.tile([128, SMR], F32)
                d1 = pp.tile([128, SMR], F32)
                nc.vector.tensor_tensor(out=d0[:], in0=z0[:], in1=m[:], op=mybir.AluOpType.subtract)
                nc.vector.tensor_tensor(out=d1[:], in0=z1[:], in1=m[:], op=mybir.AluOpType.subtract)
                e0 = pp.tile([128, SMR], F32)
                e1 = pp.tile([128, SMR], F32)
                nc.scalar.activation(e0[:], d0[:], mybir.ActivationFunctionType.Exp)
                nc.scalar.activation(e1[:], d1[:], mybir.ActivationFunctionType.Exp)
                nc.vector.tensor_tensor(out=e0[:], in0=e0[:], in1=e1[:], op=mybir.AluOpType.add)
                ls = pp.tile([128, SMR], F32)
                nc.scalar.activation(ls[:], e0[:], mybir.ActivationFunctionType.Ln)
                nc.vector.tensor_tensor(out=d0[:], in0=d0[:], in1=ls[:], op=mybir.AluOpType.subtract)
                nc.vector.tensor_tensor(out=d1[:], in0=d1[:], in1=ls[:], op=mybir.AluOpType.subtract)
                nc.sync.dma_start(out=out_d[0], in_=d0[:])
                nc.sync.dma_start(out=out_d[1], in_=d1[:])
    nc.compile()
    return nc


# ------------------------------------------------------------- preprocessing
def _preprocess(edge_index, N, SH, DST_CH, NCHUNK):
    src = np.asarray(edge_index[0]).astype(np.int64)
    dst = np.asarray(edge_index[1]).astype(np.int64)
    E = src.shape[0]
    deg = (np.bincount(dst, minlength=N) + 1).astype(np.int32)

    core = (dst // SH).astype(np.int64)
    grp = (src // SH).astype(np.int64)
    order = np.lexsort((dst, grp, core))
    s_s = src[order]
    d_s = dst[order]
    c_s = core[order]
    g_s = grp[order]
    chunk = ((d_s % SH) // DST_CH).astype(np.int64)
    bucket = (c_s * NG + g_s) * NCHUNK + chunk
    nb = NCORES * NG * NCHUNK
    counts = np.bincount(bucket, minlength=nb)
    C = int(_cdiv(max(int(counts.max()), 16), 16) * 16)
    offs = np.zeros(nb + 1, np.int64)
    np.cumsum(counts, out=offs[1:])
    pos = np.arange(E, dtype=np.int64) - offs[bucket]

    first = np.ones(E, bool)
    first[1:] = (d_s[1:] != d_s[:-1]) | (bucket[1:] != bucket[:-1])
    last = np.ones(E, bool)
    last[:-1] = first[1:]

    idx_all = np.full((NCORES, 128, NCHUNK * (C // 16)), SH, np.int16)
    p_part = (16 * g_s + pos % 16).astype(np.int64)
    p_col = (chunk * (C // 16) + pos // 16).astype(np.int64)
    idx_all[c_s, p_part, p_col] = (s_s - g_s * SH).astype(np.int16)

    maskg = np.ones((NCORES, NG, NCHUNK * C), np.float32)
    maskg[c_s[first], g_s[first], (chunk[first] * C + pos[first])] = 0.0
    mask_all = np.repeat(maskg, 16, axis=1).astype(ml_dtypes.bfloat16)

    ext_all = np.zeros((NCORES, 128, NCHUNK * (DST_CH // 16)), np.int16)
    le = np.nonzero(last)[0]
    slot = (d_s[le] % SH) % DST_CH
    ext_all[c_s[le], (16 * g_s[le] + slot % 16), (chunk[le] * (DST_CH // 16) + slot // 16)] = \
        (pos[le] + 1).astype(np.int16)

    return deg, C, idx_all, mask_all, ext_all


# ---------------------------------------------------------------------- main
def kernel(x, edge_index, W1, b1, W2, b2):
    global LAST_EXEC_NS
    LAST_EXEC_NS = []
    x = np.asarray(x, np.float32)
    W1 = np.asarray(W1, np.float32)
    b1 = np.asarray(b1, np.float32)
    W2 = np.asarray(W2, np.float32)
    b2 = np.asarray(b2, np.float32)
    N, IN = x.shape
    HID = W1.shape[1]
    OUT = W2.shape[1]
    assert N % NCORES == 0
    SH = N // NCORES
    SH_PAD = _cdiv(SH, 128) * 128
    W = SH_PAD  # gather-table columns; col SH.. are zero (pad sentinel = SH)
    DST_CH = 1024 if SH >= 1024 else max(512, _cdiv(SH, 512) * 512)
    NCHUNK = _cdiv(SH, DST_CH)
    DST_PAD = NCHUNK * DST_CH
    assert DST_PAD % 128 == 0
    trace = bool(os.environ.get("BASS_TRACE"))

    deg, C, idx_all, mask_all, ext_all = _preprocess(edge_index, N, SH, DST_CH, NCHUNK)

    # ---- launch A: per-shard h1sT = dis * (x @ W1)^T
    key_a = ("A", SH_PAD, IN, HID)
    if key_a not in _cache:
        _cache[key_a] = _build_launch_a(SH_PAD, IN, HID)
    nc_a = _cache[key_a]
    NTILE = SH_PAD // 128
    in_maps = []
    for c in range(NCORES):
        xs = np.zeros((SH_PAD, IN), np.float32)
        xs[:SH] = x[c * SH:(c + 1) * SH]
        dsh = np.ones(SH_PAD, np.int32)
        dsh[:SH] = deg[c * SH:(c + 1) * SH]
        in_maps.append({"xs": xs, "deg2d": np.ascontiguousarray(dsh.reshape(NTILE, 128).T),
                        "w1": W1})
    res_a = run_bass_kernel_spmd(nc_a, in_maps, list(range(NCORES)), trace=trace)
    LAST_EXEC_NS.append(res_a.exec_time_ns)
    h1sT = [res_a.results[c]["h1sT"] for c in range(NCORES)]          # [HID, SH_PAD]
    disf = [np.ascontiguousarray(res_a.results[c]["dis2d"].T).reshape(-1)
            for c in range(NCORES)]                                    # [SH_PAD]

    # ---- assemble shared/derived host arrays
    tableB = np.zeros((128, W), np.float32)
    for j in range(NG):
        tableB[16 * j:16 * j + HID, :] = h1sT[j]
        tableB[16 * j:16 * j + HID, SH:] = 0.0
    g1 = np.zeros((128, HID), np.float32)
    for j in range(NG):
        g1[16 * j + np.arange(HID), np.arange(HID)] = 1.0
    g2 = np.zeros((128, OUT), np.float32)
    for j in range(NG):
        g2[16 * j + np.arange(OUT), np.arange(OUT)] = 1.0

    def disrep(c, F):
        d = np.ones(DST_PAD, np.float32)
        d[:SH] = disf[c][:SH]
        return np.tile(d[None, :], (F, 1))

    # ---- launch B: layer-1 aggregation + relu + W2 -> z shards
    key_b = ("B", W, C, NCHUNK, DST_CH, DST_PAD, HID, OUT, 1)
    if key_b not in _cache:
        _cache[key_b] = _build_launch_agg(W, C, NCHUNK, DST_CH, DST_PAD, HID, OUT, 1)
    nc_b = _cache[key_b]
    in_maps = []
    for c in range(NCORES):
        selfv = np.zeros((HID, DST_PAD), np.float32)
        selfv[:, :SH] = h1sT[c][:, :SH]
        in_maps.append({
            "table": tableB, "idx": idx_all[c], "mask": mask_all[c], "ext": ext_all[c],
            "disrep": disrep(c, HID), "selfv": selfv,
            "bias": b1.reshape(HID, 1), "gmat": g1, "w2": W2,
        })
    res_b = run_bass_kernel_spmd(nc_b, in_maps, list(range(NCORES)), trace=trace)
    LAST_EXEC_NS.append(res_b.exec_time_ns)
    zs = [res_b.results[c]["z"] for c in range(NCORES)]               # [OUT, DST_PAD]

    # ---- launch C: layer-2 aggregation + bias + log_softmax
    tableC = np.zeros((128, W), np.float32)
    for j in range(NG):
        tableC[16 * j:16 * j + OUT, :SH] = zs[j][:, :SH]
    key_c = ("C", W, C, NCHUNK, DST_CH, DST_PAD, HID, OUT, 2)
    if key_c not in _cache:
        _cache[key_c] = _build_launch_agg(W, C, NCHUNK, DST_CH, DST_PAD, HID, OUT, 2)
    nc_c = _cache[key_c]
    in_maps = []
    for c in range(NCORES):
        selfv = np.zeros((OUT, DST_PAD), np.float32)
        selfv[:, :SH] = zs[c][:, :SH]
        in_maps.append({
            "table": tableC, "idx": idx_all[c], "mask": mask_all[c], "ext": ext_all[c],
            "disrep": disrep(c, OUT), "selfv": selfv,
            "bias": b2.reshape(OUT, 1), "gmat": g2,
        })
    res_c = run_bass_kernel_spmd(nc_c, in_maps, list(range(NCORES)), trace=trace)
    LAST_EXEC_NS.append(res_c.exec_time_ns)

    out = np.empty((N, OUT), np.float32)
    for c in range(NCORES):
        o = res_c.results[c]["o"].reshape(OUT, DST_PAD)
        out[c * SH:(c + 1) * SH] = o[:, :SH].T
    return out


# revision 6
# speedup vs baseline: 1.0150x; 1.0150x over previous
"""2-layer GCN (GCNConv -> relu -> GCNConv -> log_softmax) on 8 NeuronCores.

Strategy (standard distributed GNN data parallel):
  - nodes partitioned into 8 contiguous shards; core c owns dst shard c
  - edges partitioned by dst owner; within a core, bucketed by
    (src-octant "group", dst-chunk) and sorted by dst
  - per-layer aggregation on device:
      table   = dis * (features)          [16 feature-partitions x 8 src-octant groups]
      gather  = gpsimd ap_gather (per-group index streams)
      segsum  = DVE segmented scan (mask resets at dst-run starts)
      extract = gpsimd ap_gather of run-end prefix values
      combine = PE matmul with a group-summing 0/1 matrix
  - self-loop term handled analytically (dis_d^2 * h), bias/relu/W2/log_softmax
    fused in the epilogue
  - halo exchange of the (tiny) transformed features between launches is done
    on the host (concat of per-core shard outputs)

All floating point math runs on device in fp32 (masks are exact 0/1 bf16).
Host only does integer graph preprocessing and data movement.
"""
import os
import sys

sys.path.insert(0, '/opt/trn_rl_repo')

import numpy as np
import ml_dtypes

from concourse import bass, bacc, mybir
import concourse.tile as tile
from concourse.masks import make_identity
from concourse.bass_utils import run_bass_kernel_spmd

F32 = mybir.dt.float32
BF16 = mybir.dt.bfloat16
I16 = mybir.dt.int16
I32 = mybir.dt.int32

NCORES = 8
NG = 8  # src-octant groups (16 partitions each)

# accumulated HW time of the launches of the most recent kernel() call
LAST_EXEC_NS = []

_cache = {}


def _cdiv(a, b):
    return (a + b - 1) // b


# ----------------------------------------------------------------- launch A
def _build_launch_a(SH_PAD, IN, HID):
    NTILE = SH_PAD // 128
    nc = bacc.Bacc("TRN2", target_bir_lowering=False, debug=False, num_devices=NCORES)
    xs_d = nc.dram_tensor("xs", [SH_PAD, IN], F32, kind="ExternalInput")
    deg_d = nc.dram_tensor("deg2d", [128, NTILE], I32, kind="ExternalInput")
    w1_d = nc.dram_tensor("w1", [IN, HID], F32, kind="ExternalInput")
    h1sT_d = nc.dram_tensor("h1sT", [HID, SH_PAD], F32, kind="ExternalOutput")
    dis_d = nc.dram_tensor("dis2d", [128, NTILE], F32, kind="ExternalOutput")

    with tile.TileContext(nc) as tc:
        with (
            tc.tile_pool(name="persist", bufs=1) as pp,
            tc.tile_pool(name="loop", bufs=3) as lp,
            tc.tile_pool(name="psum", bufs=3, space="PSUM") as psp,
            tc.tile_pool(name="psum2", bufs=2, space="PSUM") as psp2,
        ):
            ident = pp.tile([128, 128], F32)
            make_identity(nc, ident[:])
            w1 = pp.tile([IN, HID], F32)
            nc.sync.dma_start(out=w1[:], in_=w1_d[:])
            degt = pp.tile([128, NTILE], I32)
            nc.sync.dma_start(out=degt[:], in_=deg_d[:])
            degf = pp.tile([128, NTILE], F32)
            nc.vector.tensor_copy(out=degf[:], in_=degt[:])
            dis = pp.tile([128, NTILE], F32)
            nc.vector.reciprocal(out=dis[:], in_=degf[:])
            nc.scalar.activation(dis[:], dis[:], mybir.ActivationFunctionType.Sqrt)
            nc.sync.dma_start(out=dis_d[:], in_=dis[:])

            h1sT = pp.tile([HID, SH_PAD], F32)
            for t in range(NTILE):
                xt = lp.tile([128, IN], F32, tag="xt")
                nc.sync.dma_start(out=xt[:], in_=xs_d[t * 128:(t + 1) * 128, :])
                nc.vector.tensor_tensor(
                    out=xt[:], in0=xt[:],
                    in1=dis[:, t:t + 1].to_broadcast([128, IN]),
                    op=mybir.AluOpType.mult)
                pT = psp.tile([128, 128], F32, tag="pT")
                nc.tensor.transpose(out=pT[:, :IN], in_=xt[:], identity=ident[:])
                xT = lp.tile([IN, 128], F32, tag="xT")
                nc.scalar.copy(out=xT[:], in_=pT[:IN, :])
                ph = psp2.tile([HID, 128], F32, tag="ph")
                nc.tensor.matmul(out=ph[:], lhsT=w1[:], rhs=xT[:],
                                 start=True, stop=True)
                nc.vector.tensor_copy(out=h1sT[:, t * 128:(t + 1) * 128], in_=ph[:])
            nc.sync.dma_start(out=h1sT_d[:], in_=h1sT[:])
    nc.compile()
    return nc


# --------------------------------------------------------------- launch B/C
def _build_launch_agg(W, C, NCHUNK, DST_CH, DST_PAD, HID, OUT, layer):
    """layer=1: combine->+self->*dis->+b1->relu->W2->*dis -> z [2, DST_PAD]
    layer=2: combine->+self->*dis->+b2 -> log_softmax -> out [2, 128, SMR]"""
    F = HID if layer == 1 else OUT
    SMR = DST_PAD // 128
    nc = bacc.Bacc("TRN2", target_bir_lowering=False, debug=False, num_devices=NCORES)
    table_d = nc.dram_tensor("table", [128, W], F32, kind="ExternalInput")
    idx_d = nc.dram_tensor("idx", [128, NCHUNK * (C // 16)], I16, kind="ExternalInput")
    mask_d = nc.dram_tensor("mask", [128, NCHUNK * C], BF16, kind="ExternalInput")
    ext_d = nc.dram_tensor("ext", [128, NCHUNK * (DST_CH // 16)], I16, kind="ExternalInput")
    disr_d = nc.dram_tensor("disrep", [F, DST_PAD], F32, kind="ExternalInput")
    self_d = nc.dram_tensor("selfv", [F, DST_PAD], F32, kind="ExternalInput")
    bias_d = nc.dram_tensor("bias", [F, 1], F32, kind="ExternalInput")
    g_d = nc.dram_tensor("gmat", [128, F], F32, kind="ExternalInput")
    if layer == 1:
        w2_d = nc.dram_tensor("w2", [HID, OUT], F32, kind="ExternalInput")
        out_d = nc.dram_tensor("z", [OUT, DST_PAD], F32, kind="ExternalOutput")
    else:
        out_d = nc.dram_tensor("o", [OUT, 128, SMR], F32, kind="ExternalOutput")

    NS = DST_CH // 512  # 512-col epilogue slices per chunk

    with tile.TileContext(nc) as tc:
        with (
            tc.tile_pool(name="persist", bufs=1) as pp,
            tc.tile_pool(name="loop", bufs=2) as lp,
            tc.tile_pool(name="big", bufs=2) as bigp,
            tc.tile_pool(name="ep", bufs=2) as ep,
            tc.tile_pool(name="epin", bufs=1) as epin,
            tc.tile_pool(name="psA", bufs=2, space="PSUM") as psA,
            tc.tile_pool(name="psB", bufs=2, space="PSUM") as psB,
            tc.tile_pool(name="dram", bufs=1, space="DRAM") as dp,
        ):
            table = pp.tile([128, W], F32)
            nc.sync.dma_start(out=table[:], in_=table_d[:])
            gmat = pp.tile([128, F], F32)
            nc.sync.dma_start(out=gmat[:], in_=g_d[:])
            bias = pp.tile([F, 1], F32)
            nc.sync.dma_start(out=bias[:], in_=bias_d[:])
            if layer == 1:
                w2 = pp.tile([HID, OUT], F32)
                nc.sync.dma_start(out=w2[:], in_=w2_d[:])
            if layer == 2:
                z2 = dp.tile([OUT, DST_PAD], F32)

            for k in range(NCHUNK):
                idx_t = lp.tile([128, C // 16], I16, tag="idx")
                nc.sync.dma_start(out=idx_t[:], in_=idx_d[:, k * (C // 16):(k + 1) * (C // 16)])
                mask_t = epin.tile([128, C], BF16, tag="mask")
                nc.sync.dma_start(out=mask_t[:], in_=mask_d[:, k * C:(k + 1) * C])
                ext_t = lp.tile([128, DST_CH // 16], I16, tag="ext")
                nc.sync.dma_start(out=ext_t[:], in_=ext_d[:, k * (DST_CH // 16):(k + 1) * (DST_CH // 16)])
                self_t = epin.tile([F, DST_CH], F32, tag="selfv")
                nc.sync.dma_start(out=self_t[:], in_=self_d[:, k * DST_CH:(k + 1) * DST_CH])
                disr_t = epin.tile([F, DST_CH], F32, tag="disr")
                nc.sync.dma_start(out=disr_t[:], in_=disr_d[:, k * DST_CH:(k + 1) * DST_CH])

                msg = bigp.tile([128, C], F32, tag="msg")
                nc.gpsimd.ap_gather(
                    out_ap=msg[:], in_ap=table[:], idxs_ap=idx_t[:],
                    channels=128, num_elems=W, d=1, num_idxs=C)
                csum = bigp.tile([128, C + 16], F32, tag="csum")
                nc.vector.memset(csum[:, 0:1], 0.0)
                nc.vector.tensor_tensor_scan(
                    out=csum[:, 1:C + 1], data0=mask_t[:], data1=msg[:],
                    initial=0.0, op0=mybir.AluOpType.mult, op1=mybir.AluOpType.add)
                extv = lp.tile([128, DST_CH], F32, tag="extv")
                nc.gpsimd.ap_gather(
                    out_ap=extv[:], in_ap=csum[:, 0:C + 1], idxs_ap=ext_t[:],
                    channels=128, num_elems=C + 1, d=1, num_idxs=DST_CH)

                for s in range(NS):
                    sl = slice(s * 512, (s + 1) * 512)
                    ps = psA.tile([F, 512], F32, tag="ps")
                    nc.tensor.matmul(out=ps[:], lhsT=gmat[:], rhs=extv[:, sl],
                                     start=True, stop=True)
                    a1 = ep.tile([F, 512], F32, tag="a1")
                    nc.vector.tensor_tensor(out=a1[:], in0=ps[:], in1=self_t[:, sl],
                                            op=mybir.AluOpType.add)
                    nc.vector.tensor_tensor(out=a1[:], in0=a1[:], in1=disr_t[:, sl],
                                            op=mybir.AluOpType.mult)
                    nc.vector.tensor_tensor(out=a1[:], in0=a1[:],
                                            in1=bias[:].to_broadcast([F, 512]),
                                            op=mybir.AluOpType.add)
                    if layer == 1:
                        nc.vector.tensor_scalar_max(a1[:], a1[:], 0.0)
                        ps2 = psB.tile([OUT, 512], F32, tag="ps2")
                        nc.tensor.matmul(out=ps2[:], lhsT=w2[:], rhs=a1[:],
                                         start=True, stop=True)
                        zt = ep.tile([OUT, 512], F32, tag="zt")
                        nc.vector.tensor_tensor(out=zt[:], in0=ps2[:],
                                                in1=disr_t[:OUT, sl],
                                                op=mybir.AluOpType.mult)
                        nc.sync.dma_start(
                            out=out_d[:, k * DST_CH + s * 512:k * DST_CH + (s + 1) * 512],
                            in_=zt[:])
                    else:
                        nc.sync.dma_start(
                            out=z2[:, k * DST_CH + s * 512:k * DST_CH + (s + 1) * 512],
                            in_=a1[:])

            if layer == 2:
                # log_softmax over the 2 classes, done in [128, SMR] layout
                z0 = pp.tile([128, SMR], F32)
                z1 = pp.tile([128, SMR], F32)
                nc.sync.dma_start(out=z0[:], in_=z2[0:1, :].rearrange('o (p f) -> (o p) f', p=128))
                nc.sync.dma_start(out=z1[:], in_=z2[1:2, :].rearrange('o (p f) -> (o p) f', p=128))
                m = pp.tile([128, SMR], F32)
                nc.vector.tensor_tensor(out=m[:], in0=z0[:], in1=z1[:], op=mybir.AluOpType.max)
                d0 = pp.tile([128, SMR], F32)
                d1 = pp.tile([128, SMR], F32)
                nc.vector.tensor_tensor(out=d0[:], in0=z0[:], in1=m[:], op=mybir.AluOpType.subtract)
                nc.vector.tensor_tensor(out=d1[:], in0=z1[:], in1=m[:], op=mybir.AluOpType.subtract)
                e0 = pp.tile([128, SMR], F32)
                e1 = pp.tile([128, SMR], F32)
                nc.scalar.activation(e0[:], d0[:], mybir.ActivationFunctionType.Exp)
                nc.scalar.activation(e1[:], d1[:], mybir.ActivationFunctionType.Exp)
                nc.vector.tensor_tensor(out=e0[:], in0=e0[:], in1=e1[:], op=mybir.AluOpType.add)
                ls = pp.tile([128, SMR], F32)
                nc.scalar.activation(ls[:], e0[:], mybir.ActivationFunctionType.Ln)
                nc.vector.tensor_tensor(out=d0[:], in0=d0[:], in1=ls[:], op=mybir.AluOpType.subtract)
                nc.vector.tensor_tensor(out=d1[:], in0=d1[:], in1=ls[:], op=mybir.AluOpType.subtract)
                nc.sync.dma_start(out=out_d[0], in_=d0[:])
                nc.sync.dma_start(out=out_d[1], in_=d1[:])
    nc.compile()
    return nc


# ------------------------------------------------------------- preprocessing
def _preprocess(edge_index, N, SH, DST_CH, NCHUNK):
    src = np.asarray(edge_index[0]).astype(np.int64)
    dst = np.asarray(edge_index[1]).astype(np.int64)
    E = src.shape[0]
    deg = (np.bincount(dst, minlength=N) + 1).astype(np.int32)

    core = (dst // SH).astype(np.int64)
    grp = (src // SH).astype(np.int64)
    order = np.lexsort((dst, grp, core))
    s_s = src[order]
    d_s = dst[order]
    c_s = core[order]
    g_s = grp[order]
    chunk = ((d_s % SH) // DST_CH).astype(np.int64)
    bucket = (c_s * NG + g_s) * NCHUNK + chunk
    nb = NCORES * NG * NCHUNK
    counts = np.bincount(bucket, minlength=nb)
    C = int(_cdiv(max(int(counts.max()), 16), 16) * 16)
    offs = np.zeros(nb + 1, np.int64)
    np.cumsum(counts, out=offs[1:])
    pos = np.arange(E, dtype=np.int64) - offs[bucket]

    first = np.ones(E, bool)
    first[1:] = (d_s[1:] != d_s[:-1]) | (bucket[1:] != bucket[:-1])
    last = np.ones(E, bool)
    last[:-1] = first[1:]

    idx_all = np.full((NCORES, 128, NCHUNK * (C // 16)), SH, np.int16)
    p_part = (16 * g_s + pos % 16).astype(np.int64)
    p_col = (chunk * (C // 16) + pos // 16).astype(np.int64)
    idx_all[c_s, p_part, p_col] = (s_s - g_s * SH).astype(np.int16)

    maskg = np.ones((NCORES, NG, NCHUNK * C), np.float32)
    maskg[c_s[first], g_s[first], (chunk[first] * C + pos[first])] = 0.0
    mask_all = np.repeat(maskg, 16, axis=1).astype(ml_dtypes.bfloat16)

    ext_all = np.zeros((NCORES, 128, NCHUNK * (DST_CH // 16)), np.int16)
    le = np.nonzero(last)[0]
    slot = (d_s[le] % SH) % DST_CH
    ext_all[c_s[le], (16 * g_s[le] + slot % 16), (chunk[le] * (DST_CH // 16) + slot // 16)] = \
        (pos[le] + 1).astype(np.int16)

    return deg, C, idx_all, mask_all, ext_all


# ---------------------------------------------------------------------- main
def kernel(x, edge_index, W1, b1, W2, b2):
    global LAST_EXEC_NS
    LAST_EXEC_NS = []
    x = np.asarray(x, np.float32)
    W1 = np.asarray(W1, np.float32)
    b1 = np.asarray(b1, np.float32)
    W2 = np.asarray(W2, np.float32)
    b2 = np.asarray(b2, np.float32)
    N, IN = x.shape
    HID = W1.shape[1]
    OUT = W2.shape[1]
    assert N % NCORES == 0
    SH = N // NCORES
    SH_PAD = _cdiv(SH, 128) * 128
    W = SH_PAD  # gather-table columns; col SH.. are zero (pad sentinel = SH)
    DST_CH = 1024 if SH >= 1024 else max(512, _cdiv(SH, 512) * 512)
    NCHUNK = _cdiv(SH, DST_CH)
    DST_PAD = NCHUNK * DST_CH
    assert DST_PAD % 128 == 0
    trace = bool(os.environ.get("BASS_TRACE"))

    deg, C, idx_all, mask_all, ext_all = _preprocess(edge_index, N, SH, DST_CH, NCHUNK)

    # ---- launch A: per-shard h1sT = dis * (x @ W1)^T
    key_a = ("A", SH_PAD, IN, HID)
    if key_a not in _cache:
        _cache[key_a] = _build_launch_a(SH_PAD, IN, HID)
    nc_a = _cache[key_a]
    NTILE = SH_PAD // 128
    in_maps = []
    for c in range(NCORES):
        xs = np.zeros((SH_PAD, IN), np.float32)
        xs[:SH] = x[c * SH:(c + 1) * SH]
        dsh = np.ones(SH_PAD, np.int32)
        dsh[:SH] = deg[c * SH:(c + 1) * SH]
        in_maps.append({"xs": xs, "deg2d": np.ascontiguousarray(dsh.reshape(NTILE, 128).T),
                        "w1": W1})
    res_a = run_bass_kernel_spmd(nc_a, in_maps, list(range(NCORES)), trace=trace)
    LAST_EXEC_NS.append(res_a.exec_time_ns)
    h1sT = [res_a.results[c]["h1sT"] for c in range(NCORES)]          # [HID, SH_PAD]
    disf = [np.ascontiguousarray(res_a.results[c]["dis2d"].T).reshape(-1)
            for c in range(NCORES)]                                    # [SH_PAD]

    # ---- assemble shared/derived host arrays
    tableB = np.zeros((128, W), np.float32)
    for j in range(NG):
        tableB[16 * j:16 * j + HID, :] = h1sT[j]
        tableB[16 * j:16 * j + HID, SH:] = 0.0
    g1 = np.zeros((128, HID), np.float32)
    for j in range(NG):
        g1[16 * j + np.arange(HID), np.arange(HID)] = 1.0
    g2 = np.zeros((128, OUT), np.float32)
    for j in range(NG):
        g2[16 * j + np.arange(OUT), np.arange(OUT)] = 1.0

    def disrep(c, F):
        d = np.ones(DST_PAD, np.float32)
        d[:SH] = disf[c][:SH]
        return np.tile(d[None, :], (F, 1))

    # ---- launch B: layer-1 aggregation + relu + W2 -> z shards
    key_b = ("B", W, C, NCHUNK, DST_CH, DST_PAD, HID, OUT, 1)
    if key_b not in _cache:
        _cache[key_b] = _build_launch_agg(W, C, NCHUNK, DST_CH, DST_PAD, HID, OUT, 1)
    nc_b = _cache[key_b]
    in_maps = []
    for c in range(NCORES):
        selfv = np.zeros((HID, DST_PAD), np.float32)
        selfv[:, :SH] = h1sT[c][:, :SH]
        in_maps.append({
            "table": tableB, "idx": idx_all[c], "mask": mask_all[c], "ext": ext_all[c],
            "disrep": disrep(c, HID), "selfv": selfv,
            "bias": b1.reshape(HID, 1), "gmat": g1, "w2": W2,
        })
    res_b = run_bass_kernel_spmd(nc_b, in_maps, list(range(NCORES)), trace=trace)
    LAST_EXEC_NS.append(res_b.exec_time_ns)
    zs = [res_b.results[c]["z"] for c in range(NCORES)]               # [OUT, DST_PAD]

    # ---- launch C: layer-2 aggregation + bias + log_softmax
    tableC = np.zeros((128, W), np.float32)
    for j in range(NG):
        tableC[16 * j:16 * j + OUT, :SH] = zs[j][:, :SH]
    key_c = ("C", W, C, NCHUNK, DST_CH, DST_PAD, HID, OUT, 2)
    if key_c not in _cache:
        _cache[key_c] = _build_launch_agg(W, C, NCHUNK, DST_CH, DST_PAD, HID, OUT, 2)
    nc_c = _cache[key_c]
    in_maps = []
    for c in range(NCORES):
        selfv = np.zeros((OUT, DST_PAD), np.float32)
        selfv[:, :SH] = zs[c][:, :SH]
        in_maps.append({
            "table": tableC, "idx": idx_all[c], "mask": mask_all[c], "ext": ext_all[c],
            "disrep": disrep(c, OUT), "selfv": selfv,
            "bias": b2.reshape(OUT, 1), "gmat": g2,
        })
    res_c = run_bass_kernel_spmd(nc_c, in_maps, list(range(NCORES)), trace=trace)
    LAST_EXEC_NS.append(res_c.exec_time_ns)

    out = np.empty((N, OUT), np.float32)
    for c in range(NCORES):
        o = res_c.results[c]["o"].reshape(OUT, DST_PAD)
        out[c * SH:(c + 1) * SH] = o[:, :SH].T
    return out
